# revision 40
# baseline (speedup 1.0000x reference)
"""Causal multi-head attention on 8 trn2 NeuronCores.

Sharding: core c -> (batch b = c//2, head-group hg = c%2).
Each head-group owns 8 of the 16 heads (512 of the 1024 embed dims after
the head split). Per core:
  - qT, kT   = (x[b] @ Wq_hg)^T (wq pre-scaled by 1/sqrt(d)), (x[b] @ Wk_hg)^T
  - v        = x[b] @ Wv_hg, packed per-head with a ones column -> va
  - scoresT  = kT.T-contract [krows, qrows] tiles; exp (-> bf16); causal mask
  - av       = va^T @ ex -> [128, qcols]: rows 0:64 context, rows 64:128 the
               softmax denominator replicated 64x (va carries 64 ones cols, so
               the broadcast falls out of the matmul for free)
  - ctxT     = av[0:64] * approx_recip(av[64:128])
  - partial  = ctxT.T @ Wproj_hg (+ bproj on hg==0 cores only)
Host: out[b] = partial(b,0) + partial(b,1).

All matmul operands are bf16 (fp32 PSUM accumulate): halves LDWEIGHTS
time and DMA traffic vs fp32r, no small-free-dim penalty. Softmax
reciprocal uses the fast custom-DVE approximation (~18 bits).
Attention runs qc-major so the output projection for the first 512 rows
overlaps the second attention half.
"""

import sys

try:
    import concourse.bass as bass  # noqa: F401
except Exception:
    sys.path.insert(0, "/opt/trn_rl_repo")

import numpy as np
import ml_dtypes

import concourse.bass as bass
import concourse.mybir as mybir
import concourse.tile as tile
from concourse import bacc
from concourse.bass_utils import run_bass_kernel_spmd

F32 = mybir.dt.float32
F32R = mybir.dt.float32r
BF16 = mybir.dt.bfloat16
AF = mybir.ActivationFunctionType
BF_NP = ml_dtypes.bfloat16

B, S, E = 4, 1024, 1024
H, D = 16, 64
HG = 2              # head groups (cores per batch)
HPG = H // HG       # 8 heads per group
EG = HPG * D        # 512 embed cols per group
P = 128
ET = E // P         # 8 embed tiles
RT = S // P         # 8 row tiles
CT = EG // P        # 4 col tiles of the group's q/k
QCH = 512           # q-chunk (moving free dim)
NQC = S // QCH      # 2 q chunks
SCALE = 1.0 / np.sqrt(D)


def _emit(nc, tc, with_bias):
    xt = nc.dram_tensor("xt", [ET, P, S], BF16, kind="ExternalInput")
    wq = nc.dram_tensor("wq", [ET, P, EG], BF16, kind="ExternalInput")
    wk = nc.dram_tensor("wk", [ET, P, EG], BF16, kind="ExternalInput")
    wv = nc.dram_tensor("wv", [ET, P, EG], BF16, kind="ExternalInput")
    wp = nc.dram_tensor("wp", [CT, P, E], BF16, kind="ExternalInput")
    bq = nc.dram_tensor("bq", [P, CT], F32, kind="ExternalInput")
    bk = nc.dram_tensor("bk", [P, CT], F32, kind="ExternalInput")
    bv = nc.dram_tensor("bv", [1, EG], BF16, kind="ExternalInput")
    bp = nc.dram_tensor("bp", [1, E], BF16, kind="ExternalInput")
    mask = nc.dram_tensor("mask", [P, P], BF16, kind="ExternalInput")
    ones_in = nc.dram_tensor("ones", [1, P], BF16, kind="ExternalInput")
    vones_in = nc.dram_tensor("vones", [P, HPG * D], BF16, kind="ExternalInput")
    out = nc.dram_tensor("out", [S, E], BF16, kind="ExternalOutput")

    with (
        tc.tile_pool(name="xt", bufs=1) as p_xt,
        tc.tile_pool(name="wqkv", bufs=1) as p_w,
        tc.tile_pool(name="wp", bufs=1) as p_wp,
        tc.tile_pool(name="qt", bufs=1) as p_qt,
        tc.tile_pool(name="kt", bufs=1) as p_kt,
        tc.tile_pool(name="vaug", bufs=1) as p_va,
        tc.tile_pool(name="ctxT", bufs=1) as p_ctx,
        tc.tile_pool(name="exps", bufs=16) as p_exp,
        tc.tile_pool(name="small", bufs=1) as p_sm,
        tc.tile_pool(name="recip", bufs=4) as p_rc,
        tc.tile_pool(name="osb", bufs=3) as p_osb,
        tc.tile_pool(name="pr", bufs=2, space="PSUM") as p_pr,
        tc.tile_pool(name="av", bufs=4, space="PSUM") as p_av,
    ):
        # ---- input loads, one contiguous DMA per tile, spread across the
        # three DMA-capable queues (sync/scalar/gpsimd) so descriptor
        # generation is not serialized on one engine ----
        xt_t = [p_xt.tile([P, S], BF16, tag=f"xt{et}", name=f"xt{et}")
                for et in range(ET)]
        wq_t = [p_w.tile([P, EG], BF16, tag=f"wq{et}", name=f"wq{et}")
                for et in range(ET)]
        wk_t = [p_w.tile([P, EG], BF16, tag=f"wk{et}", name=f"wk{et}")
                for et in range(ET)]
        wv_t = [p_w.tile([P, EG], BF16, tag=f"wv{et}", name=f"wv{et}")
                for et in range(ET)]
        wp_t = [p_wp.tile([P, E], BF16, tag=f"wp{et}", name=f"wp{et}")
                for et in range(CT)]

        nc.scalar.dma_start(wv_t[0][:], wv[0])
        nc.sync.dma_start(xt_t[0][:, 0:2 * P], xt[0][:, 0:2 * P])
        nc.sync.dma_start(xt_t[0][:, 2 * P:S], xt[0][:, 2 * P:S])
        for et in range(1, ET):
            nc.sync.dma_start(xt_t[et][:], xt[et])
            nc.scalar.dma_start(wv_t[et][:], wv[et])
        for et in range(ET):
            nc.sync.dma_start(wk_t[et][:], wk[et])
            nc.scalar.dma_start(wq_t[et][:], wq[et])
        for et in range(CT):
            nc.gpsimd.dma_start(wp_t[et][:], wp[et])

        mask_sb = p_sm.tile([P, P], BF16, tag="mask", name="maskt")
        nc.gpsimd.dma_start(mask_sb[:], mask[:])
        vones_sb = p_sm.tile([P, HPG * D], BF16, tag="vones", name="vones")
        nc.gpsimd.dma_start(vones_sb[:], vones_in[:])
        if with_bias:
            ones_sb = p_sm.tile([1, P], BF16, tag="ones", name="ones")
            nc.sync.dma_start(ones_sb[:], ones_in[:])
            bq_sb = p_sm.tile([P, CT], F32, tag="bq", name="bqt")
            nc.sync.dma_start(bq_sb[:], bq[:])
            bk_sb = p_sm.tile([P, CT], F32, tag="bk", name="bkt")
            nc.sync.dma_start(bk_sb[:], bk[:])
            bv_sb = p_sm.tile([1, EG], BF16, tag="bv", name="bvt")
            nc.sync.dma_start(bv_sb[:], bv[:])
            bp_sb = p_sm.tile([1, E], BF16, tag="bp", name="bpt")
            nc.sync.dma_start(bp_sb[:], bp[:])
        else:
            ones_sb = bq_sb = bk_sb = bv_sb = bp_sb = None

        # ---- v FIRST (et-outer over 8 PSUM tiles): the very first matmul
        # only needs xt0+wv0 (384KB) on-chip, so compute starts as soon as
        # DMA delivers one tile pair and never outruns the stream.
        # v natural [rows, cols], packed into va [rows, 8*(64+64)] bf16: per
        # head 64 v columns then 64 ones columns, so the av matmul emits the
        # softmax denominator replicated across 64 partitions for free.
        va_t = []
        v_out = []
        vpairs = [p_pr.tile([P, 2 * QCH], F32, tag="pr", name="pr")
                  for _ in range(2)]
        for rt in range(RT):
            va = p_va.tile([P, HPG * 2 * D], BF16, tag=f"va{rt}", name=f"va{rt}")
            va3 = va[:].rearrange("p (h d) -> p h d", h=HPG)
            nc.vector.tensor_copy(
                va3[:, :, D:2 * D],
                vones_sb[:].rearrange("p (h d) -> p h d", h=HPG))
            va_t.append(va)
            if rt < 4:
                v_out.append(vpairs[rt // 2][:, (rt % 2) * EG:(rt % 2 + 1) * EG])
            else:
                v_out.append(p_av.tile([P, EG], F32, tag="av", name="av")[:])
        for et in range(ET):
            for rt in range(RT):
                nc.tensor.matmul(
                    v_out[rt],
                    xt_t[et][:, rt * P:(rt + 1) * P],
                    wv_t[et][:],
                    start=(et == 0),
                    stop=(et == ET - 1 and not with_bias),
                )
        for rt in range(RT):
            if with_bias:
                nc.tensor.matmul(
                    v_out[rt], ones_sb[0:1, 0:P], bv_sb[0:1, :],
                    start=False, stop=True,
                )
            # one strided cast copy: [128,(8,64)] f32 -> [128,(8,128)[0:64]]
            nc.vector.tensor_copy(
                va_t[rt][:].rearrange("p (h d) -> p h d", h=HPG)[:, :, 0:D],
                v_out[rt].rearrange("p (h d) -> p h d", h=HPG))

        # ---- qT/kT [cols, rows] bf16, et-outer with 4 live PSUM tiles ----
        qT_t = [p_qt.tile([P, S], BF16, tag=f"qt{ct}", name=f"qt{ct}") for ct in range(CT)]
        kT_t = [p_kt.tile([P, S], BF16, tag=f"kt{ct}", name=f"kt{ct}") for ct in range(CT)]
        for ct in range(CT):
            # q accumulates in a 2-bank pair (one wide copy out); k in two
            # p_av singles — each pool keeps one ct of lookahead.
            qpr = p_pr.tile([P, 2 * QCH], F32, tag="pr", name="pr")
            kps = [p_av.tile([P, QCH], F32, tag="av", name="av")
                   for _ in range(NQC)]
            outs = [qpr[:, 0:QCH], qpr[:, QCH:2 * QCH], kps[0][:], kps[1][:]]
            for et in range(ET):
                for i, (wt, rc) in enumerate(
                        [(w, r) for w in (wq_t, wk_t) for r in range(NQC)]):
                    nc.tensor.matmul(
                        outs[i],
                        wt[et][:, ct * P:(ct + 1) * P],
                        xt_t[et][:, rc * QCH:(rc + 1) * QCH],
                        start=(et == 0), stop=(et == ET - 1),
                    )
            qk_copies = [
                (qT_t[ct][:, 0:S], qpr[:], bq_sb),
                (kT_t[ct][:, 0:QCH], kps[0][:], bk_sb),
                (kT_t[ct][:, QCH:S], kps[1][:], bk_sb),
            ]
            for dst, src, bias in qk_copies:
                if with_bias:
                    nc.scalar.activation(
                        dst, src, AF.Identity,
                        bias=bias[:, ct:ct + 1], scale=1.0)
                else:
                    nc.scalar.copy(dst, src)

        # ---- attention (qc-major so qc=0 ctx finishes early) ----
        ctx_t = [p_ctx.tile([P, S], BF16, tag=f"cx{i}", name=f"cx{i}") for i in range(CT)]

        def attention(qc, h):
            hp, hb = h // 2, (h % 2) * D     # tile index / partition base
            av = p_av.tile([P, QCH], F32, tag="av", name="av")
            n_kt = (qc + 1) * (QCH // P)
            # score tiles go into adjacent-bank PSUM pairs so ONE exp
            # activation covers two k-tiles (engines read across banks; only
            # matmul outputs are bank-limited). All scores + exps first, then
            # the av accumulation chain, which then never stalls on scalar.
            exs = []
            for kp in range(n_kt // 2):
                pr = p_pr.tile([P, 2 * QCH], F32, tag="pr", name="pr")
                ex = p_exp.tile([P, 2 * QCH], BF16, tag="ex", name="ex")
                w = 0
                for half in range(2):
                    kt = 2 * kp + half
                    off = max(0, (kt - qc * (QCH // P))) * P
                    n = QCH - off
                    nc.tensor.matmul(
                        pr[:, half * QCH:half * QCH + n],
                        kT_t[hp][hb:hb + D, kt * P:(kt + 1) * P],
                        qT_t[hp][hb:hb + D, qc * QCH + off:(qc + 1) * QCH],
                        start=True, stop=True,
                        tile_position=(hb, 0),
                    )
                    exs.append((ex[:, half * QCH:half * QCH + n], off, n))
                    w = half * QCH + n
                nc.scalar.activation(ex[:, 0:w], pr[:, 0:w], AF.Exp)
                # halves whose k-tile intersects the diagonal: mask the first
                # P columns of the exp'd half
                for half in range(2):
                    if 2 * kp + half >= qc * (QCH // P):
                        nc.vector.tensor_mul(
                            ex[:, half * QCH:half * QCH + P],
                            ex[:, half * QCH:half * QCH + P], mask_sb[:])
            for kt, (ex_ap, off, n) in enumerate(exs):
                nc.tensor.matmul(
                    av[:, off:QCH],
                    va_t[kt][:, h * 2 * D:(h + 1) * 2 * D],
                    ex_ap,
                    start=(kt == 0), stop=(kt == n_kt - 1),
                )
            den_sb = p_rc.tile([D, QCH], F32, tag="den", name="den")
            nc.vector.tensor_copy(den_sb[:], av[D:2 * D, :])
            rcb = p_rc.tile([D, QCH], F32, tag="rc", name="rc")
            nc.vector.reciprocal_approx_fast(rcb[:], den_sb[:])
            nc.vector.tensor_mul(
                ctx_t[hp][hb:hb + D, qc * QCH:(qc + 1) * QCH],
                av[0:D, :], rcb[:])

        # ---- output projection for one row tile ----
        def project(rt):
            for cc in range(E // QCH):
                ps = p_av.tile([P, QCH], F32, tag="av", name="av")
                for et in range(CT):
                    nc.tensor.matmul(
                        ps[:],
                        ctx_t[et][:, rt * P:(rt + 1) * P],
                        wp_t[et][:, cc * QCH:(cc + 1) * QCH],
                        start=(et == 0),
                        stop=(et == CT - 1 and not with_bias),
                    )
                if with_bias:
                    nc.tensor.matmul(
                        ps[:], ones_sb[0:1, 0:P],
                        bp_sb[0:1, cc * QCH:(cc + 1) * QCH],
                        start=False, stop=True,
                    )
                osb = p_osb.tile([P, QCH], BF16, tag="osb", name="osb")
                if (rt + cc) % 2 == 0:
                    nc.scalar.copy(osb[:], ps[:])
                else:
                    nc.vector.tensor_copy(osb[:], ps[:])
                eng = nc.sync if (rt + cc) % 2 == 0 else nc.gpsimd
                eng.dma_start(
                    out[rt * P:(rt + 1) * P, cc * QCH:(cc + 1) * QCH],
                    osb[:])

        for h in range(HPG):
            attention(0, h)
        attention(1, 0)
        # projection of the first row half only needs qc=0 ctx; emitting it
        # here gives the tensor queue work while the last qc=0 ctx drains.
        for rt in range(RT // 2):
            project(rt)
        for h in range(1, HPG):
            attention(1, h)
        for rt in range(RT // 2, RT):
            project(rt)


def build_nc(with_bias=False):
    nc = bacc.Bacc("TRN2", target_bir_lowering=False, debug=False)
    with tile.TileContext(nc) as tc, nc.allow_low_precision(
        reason="bf16 matmul operands with fp32 accumulate; approx reciprocal"
    ):
        _emit(nc, tc, with_bias)
    nc.compile()
    return nc


def make_in_maps(x, Wqkv, bqkv, Wproj, bproj):
    x = np.asarray(x, dtype=np.float32)
    Wqkv = np.asarray(Wqkv, dtype=np.float32)
    bqkv = np.asarray(bqkv, dtype=np.float32)
    Wproj = np.asarray(Wproj, dtype=np.float32)
    bproj = np.asarray(bproj, dtype=np.float32)
    mask = np.triu(np.ones((P, P), dtype=np.float32))  # [k, q]: k <= q
    in_maps = []
    for c in range(8):
        b, hg = c // 2, c % 2
        g = slice(hg * EG, (hg + 1) * EG)
        in_maps.append({
            "xt": np.ascontiguousarray(x[b].T).reshape(ET, P, S).astype(BF_NP),
            "wq": np.ascontiguousarray(
                Wqkv[:, 0 * E:1 * E][:, g] * SCALE).reshape(ET, P, EG).astype(BF_NP),
            "wk": np.ascontiguousarray(
                Wqkv[:, 1 * E:2 * E][:, g]).reshape(ET, P, EG).astype(BF_NP),
            "wv": np.ascontiguousarray(
                Wqkv[:, 2 * E:3 * E][:, g]).reshape(ET, P, EG).astype(BF_NP),
            "wp": np.ascontiguousarray(Wproj[g, :]).reshape(CT, P, E).astype(BF_NP),
            "bq": np.ascontiguousarray(
                (bqkv[0 * E:1 * E][g] * SCALE).reshape(CT, P).T),
            "bk": np.ascontiguousarray(
                bqkv[1 * E:2 * E][g].reshape(CT, P).T),
            "bv": bqkv[2 * E:3 * E][g].reshape(1, EG).astype(BF_NP),
            "bp": (bproj if hg == 0 else np.zeros_like(bproj)
                   ).reshape(1, E).astype(BF_NP),
            "mask": mask.astype(BF_NP),
            "ones": np.ones((1, P), dtype=BF_NP),
            "vones": np.ones((P, HPG * D), dtype=BF_NP),
        })
    return in_maps


def kernel(x, Wqkv, bqkv, Wproj, bproj):
    with_bias = bool(
        np.any(np.asarray(bqkv)) or np.any(np.asarray(bproj)))
    nc = build_nc(with_bias)
    in_maps = make_in_maps(x, Wqkv, bqkv, Wproj, bproj)
    res = run_bass_kernel_spmd(nc, in_maps, list(range(8))).results
    out = np.zeros((B, S, E), dtype=np.float32)
    for c in range(8):
        out[c // 2] += res[c]["out"]
    return out


# revision 41
# speedup vs baseline: 1.0821x; 1.0821x over previous
"""Causal multi-head attention on 8 trn2 NeuronCores.

Sharding: core c -> (batch b = c//2, head-group hg = c%2).
Each head-group owns 8 of the 16 heads (512 of the 1024 embed dims after
the head split). Per core:
  - qT, kT   = (x[b] @ Wq_hg)^T (wq pre-scaled by 1/sqrt(d)), (x[b] @ Wk_hg)^T
  - v        = x[b] @ Wv_hg, packed per-head with a ones column -> va
  - scoresT  = kT.T-contract [krows, qrows] tiles; exp (-> bf16); causal mask
  - av       = va^T @ ex -> [128, qcols]: rows 0:64 context, rows 64:128 the
               softmax denominator replicated 64x (va carries 64 ones cols, so
               the broadcast falls out of the matmul for free)
  - ctxT     = av[0:64] * approx_recip(av[64:128])
  - partial  = ctxT.T @ Wproj_hg (+ bproj on hg==0 cores only)
Host: out[b] = partial(b,0) + partial(b,1).

All matmul operands are bf16 (fp32 PSUM accumulate): halves LDWEIGHTS
time and DMA traffic vs fp32r, no small-free-dim penalty. Softmax
reciprocal uses the fast custom-DVE approximation (~18 bits).
Attention runs qc-major so the output projection for the first 512 rows
overlaps the second attention half.
"""

import sys

try:
    import concourse.bass as bass  # noqa: F401
except Exception:
    sys.path.insert(0, "/opt/trn_rl_repo")

import numpy as np
import ml_dtypes

import concourse.bass as bass
import concourse.mybir as mybir
import concourse.tile as tile
from concourse import bacc
from concourse.bass_utils import run_bass_kernel_spmd

F32 = mybir.dt.float32
F32R = mybir.dt.float32r
BF16 = mybir.dt.bfloat16
AF = mybir.ActivationFunctionType
BF_NP = ml_dtypes.bfloat16

B, S, E = 4, 1024, 1024
H, D = 16, 64
HG = 2              # head groups (cores per batch)
HPG = H // HG       # 8 heads per group
EG = HPG * D        # 512 embed cols per group
P = 128
ET = E // P         # 8 embed tiles
RT = S // P         # 8 row tiles
CT = EG // P        # 4 col tiles of the group's q/k
QCH = 512           # q-chunk (moving free dim)
NQC = S // QCH      # 2 q chunks
SCALE = 1.0 / np.sqrt(D)


def _emit(nc, tc, with_bias):
    xt = nc.dram_tensor("xt", [ET, P, S], BF16, kind="ExternalInput")
    wq = nc.dram_tensor("wq", [ET, P, EG], BF16, kind="ExternalInput")
    wk = nc.dram_tensor("wk", [ET, P, EG], BF16, kind="ExternalInput")
    wv = nc.dram_tensor("wv", [ET, P, EG], BF16, kind="ExternalInput")
    wp = nc.dram_tensor("wp", [CT, P, E], BF16, kind="ExternalInput")
    bq = nc.dram_tensor("bq", [P, CT], F32, kind="ExternalInput")
    bk = nc.dram_tensor("bk", [P, CT], F32, kind="ExternalInput")
    bv = nc.dram_tensor("bv", [1, EG], BF16, kind="ExternalInput")
    bp = nc.dram_tensor("bp", [1, E], BF16, kind="ExternalInput")
    mask = nc.dram_tensor("mask", [P, P], BF16, kind="ExternalInput")
    ones_in = nc.dram_tensor("ones", [1, P], BF16, kind="ExternalInput")
    vones_in = nc.dram_tensor("vones", [P, HPG * D], BF16, kind="ExternalInput")
    out = nc.dram_tensor("out", [S, E], BF16, kind="ExternalOutput")

    with (
        tc.tile_pool(name="xt", bufs=1) as p_xt,
        tc.tile_pool(name="wqkv", bufs=1) as p_w,
        tc.tile_pool(name="wp", bufs=1) as p_wp,
        tc.tile_pool(name="qt", bufs=1) as p_qt,
        tc.tile_pool(name="kt", bufs=1) as p_kt,
        tc.tile_pool(name="vaug", bufs=1) as p_va,
        tc.tile_pool(name="ctxT", bufs=1) as p_ctx,
        tc.tile_pool(name="exps", bufs=16) as p_exp,
        tc.tile_pool(name="small", bufs=1) as p_sm,
        tc.tile_pool(name="recip", bufs=4) as p_rc,
        tc.tile_pool(name="osb", bufs=3) as p_osb,
        tc.tile_pool(name="pr", bufs=2, space="PSUM") as p_pr,
        tc.tile_pool(name="av", bufs=4, space="PSUM") as p_av,
    ):
        # ---- input loads, one contiguous DMA per tile, spread across the
        # three DMA-capable queues (sync/scalar/gpsimd) so descriptor
        # generation is not serialized on one engine ----
        xt_t = [p_xt.tile([P, S], BF16, tag=f"xt{et}", name=f"xt{et}")
                for et in range(ET)]
        wq_t = [p_w.tile([P, EG], BF16, tag=f"wq{et}", name=f"wq{et}")
                for et in range(ET)]
        wk_t = [p_w.tile([P, EG], BF16, tag=f"wk{et}", name=f"wk{et}")
                for et in range(ET)]
        wv_t = [p_w.tile([P, EG], BF16, tag=f"wv{et}", name=f"wv{et}")
                for et in range(ET)]
        wp_t = [p_wp.tile([P, E], BF16, tag=f"wp{et}", name=f"wp{et}")
                for et in range(CT)]

        nc.scalar.dma_start(wv_t[0][:], wv[0])
        nc.sync.dma_start(xt_t[0][:, 0:2 * P], xt[0][:, 0:2 * P])
        nc.sync.dma_start(xt_t[0][:, 2 * P:S], xt[0][:, 2 * P:S])
        for et in range(1, ET):
            nc.sync.dma_start(xt_t[et][:], xt[et])
            nc.scalar.dma_start(wv_t[et][:], wv[et])
        mask_sb = p_sm.tile([P, P], BF16, tag="mask", name="maskt")
        nc.gpsimd.dma_start(mask_sb[:], mask[:])
        vones_sb = p_sm.tile([P, HPG * D], BF16, tag="vones", name="vones")
        nc.gpsimd.dma_start(vones_sb[:], vones_in[:])
        for et in range(ET):
            nc.sync.dma_start(wk_t[et][:], wk[et])
            nc.scalar.dma_start(wq_t[et][:], wq[et])
        for et in range(CT):
            nc.gpsimd.dma_start(wp_t[et][:], wp[et])
        if with_bias:
            ones_sb = p_sm.tile([1, P], BF16, tag="ones", name="ones")
            nc.sync.dma_start(ones_sb[:], ones_in[:])
            bq_sb = p_sm.tile([P, CT], F32, tag="bq", name="bqt")
            nc.sync.dma_start(bq_sb[:], bq[:])
            bk_sb = p_sm.tile([P, CT], F32, tag="bk", name="bkt")
            nc.sync.dma_start(bk_sb[:], bk[:])
            bv_sb = p_sm.tile([1, EG], BF16, tag="bv", name="bvt")
            nc.sync.dma_start(bv_sb[:], bv[:])
            bp_sb = p_sm.tile([1, E], BF16, tag="bp", name="bpt")
            nc.sync.dma_start(bp_sb[:], bp[:])
        else:
            ones_sb = bq_sb = bk_sb = bv_sb = bp_sb = None

        # ---- v FIRST (et-outer over 8 PSUM tiles): the very first matmul
        # only needs xt0+wv0 (384KB) on-chip, so compute starts as soon as
        # DMA delivers one tile pair and never outruns the stream.
        # v natural [rows, cols], packed into va [rows, 8*(64+64)] bf16: per
        # head 64 v columns then 64 ones columns, so the av matmul emits the
        # softmax denominator replicated across 64 partitions for free.
        va_t = []
        v_out = []
        vpairs = [p_pr.tile([P, 2 * QCH], F32, tag="pr", name="pr")
                  for _ in range(2)]
        for rt in range(RT):
            va = p_va.tile([P, HPG * 2 * D], BF16, tag=f"va{rt}", name=f"va{rt}")
            va3 = va[:].rearrange("p (h d) -> p h d", h=HPG)
            nc.vector.tensor_copy(
                va3[:, :, D:2 * D],
                vones_sb[:].rearrange("p (h d) -> p h d", h=HPG))
            va_t.append(va)
            if rt < 4:
                v_out.append(vpairs[rt // 2][:, (rt % 2) * EG:(rt % 2 + 1) * EG])
            else:
                v_out.append(p_av.tile([P, EG], F32, tag="av", name="av")[:])
        for et in range(ET):
            for rt in range(RT):
                nc.tensor.matmul(
                    v_out[rt],
                    xt_t[et][:, rt * P:(rt + 1) * P],
                    wv_t[et][:],
                    start=(et == 0),
                    stop=(et == ET - 1 and not with_bias),
                )
        for rt in range(RT):
            if with_bias:
                nc.tensor.matmul(
                    v_out[rt], ones_sb[0:1, 0:P], bv_sb[0:1, :],
                    start=False, stop=True,
                )
            # one strided cast copy: [128,(8,64)] f32 -> [128,(8,128)[0:64]]
            nc.vector.tensor_copy(
                va_t[rt][:].rearrange("p (h d) -> p h d", h=HPG)[:, :, 0:D],
                v_out[rt].rearrange("p (h d) -> p h d", h=HPG))

        # ---- qT/kT [cols, rows] bf16, et-outer with 4 live PSUM tiles ----
        qT_t = [p_qt.tile([P, S], BF16, tag=f"qt{ct}", name=f"qt{ct}") for ct in range(CT)]
        kT_t = [p_kt.tile([P, S], BF16, tag=f"kt{ct}", name=f"kt{ct}") for ct in range(CT)]
        for ct in range(CT):
            # q accumulates in a 2-bank pair (one wide copy out); k in two
            # p_av singles — each pool keeps one ct of lookahead.
            qpr = p_pr.tile([P, 2 * QCH], F32, tag="pr", name="pr")
            kps = [p_av.tile([P, QCH], F32, tag="av", name="av")
                   for _ in range(NQC)]
            outs = [qpr[:, 0:QCH], qpr[:, QCH:2 * QCH], kps[0][:], kps[1][:]]
            for et in range(ET):
                for i, (wt, rc) in enumerate(
                        [(w, r) for w in (wq_t, wk_t) for r in range(NQC)]):
                    nc.tensor.matmul(
                        outs[i],
                        wt[et][:, ct * P:(ct + 1) * P],
                        xt_t[et][:, rc * QCH:(rc + 1) * QCH],
                        start=(et == 0), stop=(et == ET - 1),
                    )
            qk_copies = [
                (qT_t[ct][:, 0:S], qpr[:], bq_sb),
                (kT_t[ct][:, 0:QCH], kps[0][:], bk_sb),
                (kT_t[ct][:, QCH:S], kps[1][:], bk_sb),
            ]
            for dst, src, bias in qk_copies:
                if with_bias:
                    nc.scalar.activation(
                        dst, src, AF.Identity,
                        bias=bias[:, ct:ct + 1], scale=1.0)
                else:
                    nc.scalar.copy(dst, src)

        # ---- attention (qc-major so qc=0 ctx finishes early) ----
        ctx_t = [p_ctx.tile([P, S], BF16, tag=f"cx{i}", name=f"cx{i}") for i in range(CT)]

        def attention(qc, h):
            hp, hb = h // 2, (h % 2) * D     # tile index / partition base
            av = p_av.tile([P, QCH], F32, tag="av", name="av")
            n_kt = (qc + 1) * (QCH // P)
            # score tiles go into adjacent-bank PSUM pairs so ONE exp
            # activation covers two k-tiles (engines read across banks; only
            # matmul outputs are bank-limited). All scores + exps first, then
            # the av accumulation chain, which then never stalls on scalar.
            exs = []
            for kp in range(n_kt // 2):
                pr = p_pr.tile([P, 2 * QCH], F32, tag="pr", name="pr")
                ex = p_exp.tile([P, 2 * QCH], BF16, tag="ex", name="ex")
                w = 0
                for half in range(2):
                    kt = 2 * kp + half
                    off = max(0, (kt - qc * (QCH // P))) * P
                    n = QCH - off
                    nc.tensor.matmul(
                        pr[:, half * QCH:half * QCH + n],
                        kT_t[hp][hb:hb + D, kt * P:(kt + 1) * P],
                        qT_t[hp][hb:hb + D, qc * QCH + off:(qc + 1) * QCH],
                        start=True, stop=True,
                        tile_position=(hb, 0),
                    )
                    exs.append((ex[:, half * QCH:half * QCH + n], off, n))
                    w = half * QCH + n
                nc.scalar.activation(ex[:, 0:w], pr[:, 0:w], AF.Exp)
                # halves whose k-tile intersects the diagonal: mask the first
                # P columns of the exp'd half
                for half in range(2):
                    if 2 * kp + half >= qc * (QCH // P):
                        nc.vector.tensor_mul(
                            ex[:, half * QCH:half * QCH + P],
                            ex[:, half * QCH:half * QCH + P], mask_sb[:])
            for kt, (ex_ap, off, n) in enumerate(exs):
                nc.tensor.matmul(
                    av[:, off:QCH],
                    va_t[kt][:, h * 2 * D:(h + 1) * 2 * D],
                    ex_ap,
                    start=(kt == 0), stop=(kt == n_kt - 1),
                )
            den_sb = p_rc.tile([D, QCH], F32, tag="den", name="den")
            nc.vector.tensor_copy(den_sb[:], av[D:2 * D, :])
            rcb = p_rc.tile([D, QCH], F32, tag="rc", name="rc")
            nc.vector.reciprocal_approx_fast(rcb[:], den_sb[:])
            nc.vector.tensor_mul(
                ctx_t[hp][hb:hb + D, qc * QCH:(qc + 1) * QCH],
                av[0:D, :], rcb[:])

        # ---- output projection for one row tile ----
        def project(rt):
            for cc in range(E // QCH):
                ps = p_av.tile([P, QCH], F32, tag="av", name="av")
                for et in range(CT):
                    nc.tensor.matmul(
                        ps[:],
                        ctx_t[et][:, rt * P:(rt + 1) * P],
                        wp_t[et][:, cc * QCH:(cc + 1) * QCH],
                        start=(et == 0),
                        stop=(et == CT - 1 and not with_bias),
                    )
                if with_bias:
                    nc.tensor.matmul(
                        ps[:], ones_sb[0:1, 0:P],
                        bp_sb[0:1, cc * QCH:(cc + 1) * QCH],
                        start=False, stop=True,
                    )
                osb = p_osb.tile([P, QCH], BF16, tag="osb", name="osb")
                if (rt + cc) % 2 == 0:
                    nc.scalar.copy(osb[:], ps[:])
                else:
                    nc.vector.tensor_copy(osb[:], ps[:])
                eng = nc.sync if (rt + cc) % 2 == 0 else nc.gpsimd
                eng.dma_start(
                    out[rt * P:(rt + 1) * P, cc * QCH:(cc + 1) * QCH],
                    osb[:])

        for h in range(HPG):
            attention(0, h)
        attention(1, 0)
        # projection of the first row half only needs qc=0 ctx; emitting it
        # here gives the tensor queue work while the last qc=0 ctx drains.
        for rt in range(RT // 2):
            project(rt)
        for h in range(1, HPG):
            attention(1, h)
        for rt in range(RT // 2, RT):
            project(rt)


def build_nc(with_bias=False):
    nc = bacc.Bacc("TRN2", target_bir_lowering=False, debug=False)
    with tile.TileContext(nc) as tc, nc.allow_low_precision(
        reason="bf16 matmul operands with fp32 accumulate; approx reciprocal"
    ):
        _emit(nc, tc, with_bias)
    nc.compile()
    return nc


def make_in_maps(x, Wqkv, bqkv, Wproj, bproj):
    x = np.asarray(x, dtype=np.float32)
    Wqkv = np.asarray(Wqkv, dtype=np.float32)
    bqkv = np.asarray(bqkv, dtype=np.float32)
    Wproj = np.asarray(Wproj, dtype=np.float32)
    bproj = np.asarray(bproj, dtype=np.float32)
    mask = np.triu(np.ones((P, P), dtype=np.float32))  # [k, q]: k <= q
    in_maps = []
    for c in range(8):
        b, hg = c // 2, c % 2
        g = slice(hg * EG, (hg + 1) * EG)
        in_maps.append({
            "xt": np.ascontiguousarray(x[b].T).reshape(ET, P, S).astype(BF_NP),
            "wq": np.ascontiguousarray(
                Wqkv[:, 0 * E:1 * E][:, g] * SCALE).reshape(ET, P, EG).astype(BF_NP),
            "wk": np.ascontiguousarray(
                Wqkv[:, 1 * E:2 * E][:, g]).reshape(ET, P, EG).astype(BF_NP),
            "wv": np.ascontiguousarray(
                Wqkv[:, 2 * E:3 * E][:, g]).reshape(ET, P, EG).astype(BF_NP),
            "wp": np.ascontiguousarray(Wproj[g, :]).reshape(CT, P, E).astype(BF_NP),
            "bq": np.ascontiguousarray(
                (bqkv[0 * E:1 * E][g] * SCALE).reshape(CT, P).T),
            "bk": np.ascontiguousarray(
                bqkv[1 * E:2 * E][g].reshape(CT, P).T),
            "bv": bqkv[2 * E:3 * E][g].reshape(1, EG).astype(BF_NP),
            "bp": (bproj if hg == 0 else np.zeros_like(bproj)
                   ).reshape(1, E).astype(BF_NP),
            "mask": mask.astype(BF_NP),
            "ones": np.ones((1, P), dtype=BF_NP),
            "vones": np.ones((P, HPG * D), dtype=BF_NP),
        })
    return in_maps


def kernel(x, Wqkv, bqkv, Wproj, bproj):
    with_bias = bool(
        np.any(np.asarray(bqkv)) or np.any(np.asarray(bproj)))
    nc = build_nc(with_bias)
    in_maps = make_in_maps(x, Wqkv, bqkv, Wproj, bproj)
    res = run_bass_kernel_spmd(nc, in_maps, list(range(8))).results
    out = np.zeros((B, S, E), dtype=np.float32)
    for c in range(8):
        out[c // 2] += res[c]["out"]
    return out
